# revision 15
# baseline (speedup 1.0000x reference)
"""Trainium2 Bass kernel for BreakthroughSNN (spiking predictive-coding net).

Strategy (8 NeuronCores, no collectives):
  - Every core redundantly runs the tiny-but-sequential recurrence
    (T*S = 512 steps, 2 layers) on the full batch B=4.
  - The huge output projection [1024,512] @ [512,50257] is sharded by
    vocab across the 8 cores; host concatenates the slices.
  - LayerNorm gains/biases are folded into the weights host-side.
    LIF membrane bias is folded into a shifted-threshold formulation.
  - On-device layout is "folded transposed": feature-on-partition,
    columns = (layer, chunk, batch), so every element-wise op covers
    both layers + all chunks + all batch lanes in one instruction.
"""
import os
import sys

sys.path.insert(0, "/opt/trn_rl_repo")

import numpy as np
import concourse.bass as bass
import concourse.mybir as mybir
import concourse.tile as tile
from concourse import bacc
from concourse.bass_utils import run_bass_kernel_spmd

F32 = mybir.dt.float32
F32R = mybir.dt.float32r
I32 = mybir.dt.int32
AL = mybir.AluOpType
AF = mybir.ActivationFunctionType
AX = mybir.AxisListType

V, D, DS, L, T, B, S = 50257, 512, 256, 2, 2, 4, 256
BETA, THR, EPS = 0.95, 1.0, 1e-5
NCORES = 8
VP = 6288            # per-core padded vocab width: 8*6288 = 50304 >= V
R = B * S            # 1024 output rows, r = b*S + s
DC = D // 128        # 4 feature chunks for D-sized tensors
DSC = DS // 128      # 2 feature chunks for DS-sized tensors
FD = L * DC * B      # 32 cols: stacked-D layout (l, c, b)
FS = L * DSC * B     # 16 cols: stacked-DS layout (l, c, b)
NBT = (VP + 511) // 512  # 13 projection column tiles


# ----------------------------------------------------------------- host prep
def _fold(inputs):
    """Fold LN gains/biases into weights; build const blobs (all fp32)."""
    gen_w = np.asarray(inputs["gen_w"], np.float32)   # [L, DS, D]
    gen_b = np.asarray(inputs["gen_b"], np.float32)   # [L, D]
    inf_w = np.asarray(inputs["inf_w"], np.float32)   # [L, D, DS]
    inf_b = np.asarray(inputs["inf_b"], np.float32)   # [L, DS]
    ns_g = np.asarray(inputs["ns_g"], np.float32)     # [L, DS]
    ns_b = np.asarray(inputs["ns_b"], np.float32)
    ne_g = np.asarray(inputs["ne_g"], np.float32)     # [L, D]
    ne_b = np.asarray(inputs["ne_b"], np.float32)
    es = np.asarray(inputs["err_scale"], np.float32)  # [L]

    gw = gen_w * ns_g[:, :, None]                     # [L, DS, D]
    gb = gen_b + np.einsum("ld,ldo->lo", ns_b, gen_w)  # [L, D]
    iw = inf_w * ne_g[:, :, None]                     # [L, D, DS]
    ib = inf_b + np.einsum("ld,ldo->lo", ne_b, inf_w)  # [L, DS]

    kg = -gb / (1.0 - BETA)                           # ghat = mem + kg
    ki = -ib / (1.0 - BETA)
    thr_g = THR + kg                                  # spike: ghat' > thr_g
    thr_i = THR + ki

    def fold_cols(x, nch):  # [L, nch*128] -> [128, L*nch] cols (l,c)
        return x.reshape(L, nch, 128).transpose(2, 0, 1).reshape(128, L * nch)

    def expand_b(x128):  # [128, K] -> [128, K*B] cols (.., b) replicated
        return np.repeat(x128, B, axis=1)

    thrGF = expand_b(fold_cols(thr_g, DC))            # [128, FD]
    ginitF = expand_b(fold_cols(kg, DC))
    thrIF = expand_b(fold_cols(thr_i, DSC))           # [128, FS]
    iinitF = expand_b(fold_cols(ki, DSC))

    # stationary weight blocks, lhsT[k, m] layout
    gw_blk = gw.reshape(L, DSC, 128, DC, 128).transpose(0, 1, 3, 2, 4)
    iw_blk = iw.reshape(L, DC, 128, DSC, 128).transpose(0, 1, 3, 2, 4)
    enc_w = np.asarray(inputs["enc_w"], np.float32)
    encw_blk = enc_w.reshape(DC, 128, DC, 128).transpose(0, 2, 1, 3)  # [k,m,128,128]
    enc_b = np.asarray(inputs["enc_b"], np.float32)

    out_w = np.asarray(inputs["out_w"], np.float32)   # [L*DS, V]
    out_b = np.asarray(inputs["out_b"], np.float32)   # [V]
    outw_pad = np.zeros((L * DS, NCORES * VP), np.float32)
    outw_pad[:, :V] = out_w
    outb_pad = np.zeros((NCORES * VP,), np.float32)
    outb_pad[:V] = out_b

    return dict(gw_blk=gw_blk, iw_blk=iw_blk, encw_blk=encw_blk, enc_b=enc_b,
                thrGF=thrGF, ginitF=ginitF, thrIF=thrIF, iinitF=iinitF,
                outw_pad=outw_pad, outb_pad=outb_pad, es=es)


class _Arena:
    """Pack all constants into one [128, W] fp32 blob -> single DMA."""

    def __init__(self):
        self.cols = []
        self.cursor = 0
        self.slots = {}

    def add(self, name, arr):  # arr [128, w] or [rows<=128, w]
        arr = np.asarray(arr, np.float32)
        if arr.ndim == 1:
            arr = arr[:, None]
        rows, w = arr.shape
        blk = np.zeros((128, w), np.float32)
        blk[:rows] = arr
        self.cols.append(blk)
        self.slots[name] = (self.cursor, w)
        self.cursor += w

    def blob(self):
        return np.concatenate(self.cols, axis=1)

    def view(self, t, name, rows=None):
        c0, w = self.slots[name]
        if rows is None:
            return t[:, c0:c0 + w]
        return t[rows[0]:rows[1], c0:c0 + w]


def _make_arena(fd):
    ar = _Arena()
    ar.add("eye", np.eye(128, dtype=np.float32))
    for l in range(L):
        for k in range(DSC):
            for m in range(DC):
                ar.add(f"gw_{l}_{k}_{m}", fd["gw_blk"][l, k, m])
    for l in range(L):
        for k in range(DC):
            for m in range(DSC):
                ar.add(f"iw_{l}_{k}_{m}", fd["iw_blk"][l, k, m])
    for k in range(DC):
        for m in range(DC):
            ar.add(f"encw_{k}_{m}", fd["encw_blk"][k, m])
    # stat stationaries (column vectors)
    ar.add("vDSn", np.full((128, 1), -1.0 / DS, np.float32))
    ar.add("vDSp", np.full((128, 1), 1.0 / DS, np.float32))
    ar.add("vDn", np.full((128, 1), -1.0 / D, np.float32))
    ar.add("vDp", np.full((128, 1), 1.0 / D, np.float32))
    # row consts on partition 0 only (matmul base-partition constraint)
    ar.add("onesRow", np.ones((1, 512), np.float32))
    ar.add("zero128", np.zeros((128, 1), np.float32))
    ar.add("eps1", np.full((1, 1), EPS, np.float32))
    for m in range(DC):
        ar.add(f"encB_{m}", fd["enc_b"][None, m * 128:(m + 1) * 128])
    ar.add("thrGF", fd["thrGF"])
    ar.add("ginitF", fd["ginitF"])
    ar.add("thrIF", fd["thrIF"])
    ar.add("iinitF", fd["iinitF"])
    return ar


# --------------------------------------------------------------- device build
def build_program(fd, krep=1, su=16, n_steps=S, n_pass=T, dbg=False,
                  emb_rows=V):
    """Build + bacc-compile the per-core program (same for all cores;
    out_w/out_b slices differ via per-core inputs)."""
    es = fd["es"]
    ar = _make_arena(fd)
    blob = ar.blob()

    nc = bacc.Bacc("TRN2", target_bir_lowering=False, debug=False,
                   num_devices=NCORES)
    embD = nc.dram_tensor("emb", [emb_rows, D], F32, kind="ExternalInput")
    idsD = nc.dram_tensor("ids", [R, 1], I32, kind="ExternalInput")
    outwD = nc.dram_tensor("outw", [128, DC, VP], F32, kind="ExternalInput")
    outbD = nc.dram_tensor("outb", [1, VP], F32, kind="ExternalInput")
    outD = nc.dram_tensor("out", [R, VP], F32, kind="ExternalOutput")
    if dbg:
        seqD = nc.dram_tensor("dbg_seq", [128, DC * R], F32, kind="ExternalOutput")
        histD = nc.dram_tensor("dbg_hist", [128, L * DSC * R], F32,
                               kind="ExternalOutput")
        ghD = nc.dram_tensor("dbg_gh", [128, FD], F32, kind="ExternalOutput")
        ihD = nc.dram_tensor("dbg_ih", [128, FS], F32, kind="ExternalOutput")
    constD = nc.inline_tensor(blob, name="consts")

    with tile.TileContext(nc) as tc:
        import contextlib
        stk = contextlib.ExitStack()
        with stk:
            big = stk.enter_context(tc.tile_pool(name="big", bufs=1))
            work = stk.enter_context(tc.tile_pool(name="work", bufs=2))
            psA = stk.enter_context(tc.tile_pool(name="psA", bufs=2, space="PSUM"))
            psB = stk.enter_context(tc.tile_pool(name="psB", bufs=2, space="PSUM"))
            psT = stk.enter_context(tc.tile_pool(name="psT", bufs=2, space="PSUM"))
            psO = stk.enter_context(tc.tile_pool(name="psO", bufs=2, space="PSUM"))

            arena = big.tile([128, blob.shape[1]], F32, tag="arena")
            nc.sync.dma_start(out=arena[:], in_=constD[:])
            av = lambda name: ar.view(arena, name)
            onesRow = ar.view(arena, "onesRow")[0:1, :]       # [1, 512] of ones
            encBrow = lambda m: ar.view(arena, f"encB_{m}")[0:1, :]

            seqF = big.tile([128, DC * R], F32, tag="seqF")    # col c*1024+b*256+s
            hist = big.tile([128, L * DSC * R], F32, tag="hist")
            embTF = big.tile([128, DC * R], F32, tag="embTF")  # col c*1024 + r
            outBsb = big.tile([1, VP], F32, tag="outBsb")
            nc.sync.dma_start(out=outBsb[:], in_=outbD[:])

            # persistent recurrence state
            stF = big.tile([128, FS], F32, tag="stF")
            ghF = big.tile([128, FD], F32, tag="ghF")
            ihF = big.tile([128, FS], F32, tag="ihF")

            def emit_all():
                # ---- phase 1: gather + transpose + encode ----
                for t in range(8):
                    idxt = work.tile([128, 1], I32, tag="idxt")
                    nc.sync.dma_start(out=idxt[:], in_=idsD[t * 128:(t + 1) * 128, :])
                    gth = work.tile([128, D], F32, tag="gth")
                    nc.gpsimd.indirect_dma_start(
                        out=gth[:], out_offset=None, in_=embD[:],
                        in_offset=bass.IndirectOffsetOnAxis(ap=idxt[:, :1], axis=0))
                    for c in range(DC):
                        tp = psT.tile([128, 512], F32, tag="pt")
                        nc.tensor.transpose(out=tp[:, 0:128],
                                            in_=gth[:, c * 128:(c + 1) * 128],
                                            identity=av("eye"))
                        nc.vector.tensor_copy(
                            out=embTF[:, c * R + t * 128:c * R + (t + 1) * 128],
                            in_=tp[:, 0:128])
                for m in range(DC):
                    for h in range(2):
                        ep = psT.tile([128, 512], F32, tag="pt")
                        for k in range(DC):
                            nc.tensor.matmul(
                                out=ep[:], lhsT=av(f"encw_{k}_{m}"),
                                rhs=embTF[:, k * R + h * 512:k * R + h * 512 + 512],
                                start=(k == 0), stop=False)
                        nc.tensor.matmul(out=ep[:], lhsT=encBrow(m),
                                         rhs=onesRow[:, 0:512],
                                         start=False, stop=True)
                        nc.vector.tensor_copy(
                            out=seqF[:, m * R + h * 512:m * R + h * 512 + 512],
                            in_=ep[:])

                # ---- phase 2: recurrence ----
                nc.vector.tensor_copy(out=ghF[:], in_=av("ginitF"))
                nc.vector.tensor_copy(out=ihF[:], in_=av("iinitF"))
                nc.vector.memset(stF[:], 0.0)

                seq4 = seqF[:].rearrange("p (c b s) -> p c b s", c=DC, b=B)
                hist4 = hist[:].rearrange("p (f b s) -> p f b s", f=L * DSC, b=B)

                for pz in range(n_pass):
                    snap = (pz == n_pass - 1)
                    with tc.For_i(0, n_steps, su) as s0:
                        for u in range(su):
                            _step(s0 + u, snap, seq4, hist4)

                if dbg:
                    nc.sync.dma_start(out=seqD[:], in_=seqF[:])
                    nc.sync.dma_start(out=histD[:], in_=hist[:])
                    nc.sync.dma_start(out=ghD[:], in_=ghF[:])
                    nc.sync.dma_start(out=ihD[:], in_=ihF[:])
                # ---- phase 3: projection ----
                histR = big.tile([128, L * DSC * R], F32R, tag="histR")
                nc.vector.tensor_copy(out=histR[:], in_=hist[:])
                outw3 = outwD  # [128, DC, VP]
                for nb in range(NBT):
                    n0 = nb * 512
                    w = min(512, VP - n0)
                    wsl = work.tile([128, DC * 512], F32, tag="wsl")
                    nc.sync.dma_start(
                        out=wsl[:].rearrange("p (c n) -> p c n", c=DC)[:, :, 0:w],
                        in_=outw3[:, :, n0:n0 + w])
                    wslR = work.tile([128, DC * 512], F32R, tag="wslR")
                    nc.vector.tensor_copy(out=wslR[:], in_=wsl[:])
                    obP = psO.tile([128, 512], F32, tag="obP")
                    nc.tensor.matmul(out=obP[:, 0:w], lhsT=onesRow[:, 0:128],
                                     rhs=outBsb[0:1, n0:n0 + w],
                                     start=True, stop=True)
                    obS = work.tile([128, 512], F32, tag="obS")
                    nc.vector.tensor_copy(out=obS[:, 0:w], in_=obP[:, 0:w])
                    for m in range(8):
                        pp = psT.tile([128, 512], F32, tag="pt")
                        for c in range(DC):
                            nc.tensor.matmul(
                                out=pp[:, 0:w],
                                lhsT=histR[:, c * R + m * 128:c * R + (m + 1) * 128],
                                rhs=wslR[:, c * 512:c * 512 + w],
                                start=(c == 0), stop=(c == DC - 1))
                        ost = work.tile([128, 512], F32, tag="ost")
                        nc.vector.tensor_tensor(out=ost[:, 0:w], in0=pp[:, 0:w],
                                                in1=obS[:, 0:w], op=AL.add)
                        nc.sync.dma_start(
                            out=outD[m * 128:(m + 1) * 128, n0:n0 + w],
                            in_=ost[:, 0:w])

            def _step(sE, snap, seq4, hist4):
                bmm = psA.tile([128, FD + FS], F32, tag="bmm")   # predP | infP
                bst = psB.tile([128, 192], F32, tag="bst")
                tiny = work.tile([1, 64], F32, tag="tiny")
                acs = work.tile([1, 32], F32, tag="acs")

                predP = bmm[:, 0:FD]
                infP = bmm[:, FD:FD + FS]
                # bst col map: [0:16) mnS, [16:32) sqS, [32:64) mnE, [64:96) sqE,
                #              [96:128) ACSx, [128:192) ACEx
                mnSp, sqSp = bst[0:1, 0:FS], bst[0:1, 16:16 + FS]
                mnEp, sqEp = bst[0:1, 32:32 + FD], bst[0:1, 64:64 + FD]
                ACSx, ACEx = bst[:, 96:96 + 2 * FS], bst[:, 128:128 + 2 * FD]
                # tiny col map
                mnS8, sqS8 = tiny[0:1, 0:8], tiny[0:1, 8:16]
                vS, sdS = tiny[0:1, 16:24], tiny[0:1, 24:32]
                mnE8, sqE8 = tiny[0:1, 32:40], tiny[0:1, 40:48]
                vE, sdE = tiny[0:1, 48:56], tiny[0:1, 56:64]
                rsS, cS = acs[0:1, 0:8], acs[0:1, 8:16]
                rsE, cE = acs[0:1, 16:24], acs[0:1, 24:32]

                # --- stats of states (DS norm), layers stacked ---
                sqSF = work.tile([128, FS], F32, tag="sqSF")
                nc.scalar.activation(sqSF[:], stF[:], AF.Square, bias=av('zero128'))
                nc.tensor.matmul(out=mnSp, lhsT=av("vDSn"), rhs=stF[:],
                                 start=True, stop=True)
                nc.tensor.matmul(out=sqSp, lhsT=av("vDSp"), rhs=sqSF[:],
                                 start=True, stop=True)
                nc.vector.tensor_reduce(
                    out=mnS8.rearrange("p (l b) -> p l b", l=L),
                    in_=mnSp.rearrange("p (l c b) -> p l b c", l=L, c=DSC),
                    axis=AX.X, op=AL.add)
                nc.vector.tensor_reduce(
                    out=sqS8.rearrange("p (l b) -> p l b", l=L),
                    in_=sqSp.rearrange("p (l c b) -> p l b c", l=L, c=DSC),
                    axis=AX.X, op=AL.add)
                msqS = work.tile([1, 8], F32, tag="msqS")
                nc.vector.tensor_tensor(out=msqS[:], in0=mnS8, in1=mnS8, op=AL.mult)
                nc.vector.tensor_tensor(out=vS, in0=sqS8, in1=msqS[:], op=AL.subtract)
                nc.scalar.activation(sdS, vS, AF.Sqrt, bias=av('eps1')[0:1, :])
                nc.vector.reciprocal(rsS, sdS)
                nc.vector.tensor_tensor(out=cS, in0=mnS8, in1=rsS, op=AL.mult)
                nc.tensor.matmul(
                    out=ACSx, lhsT=onesRow[:, 0:128],
                    rhs=acs[0:1, 0:16].rearrange("p (a l b) -> p a l b", a=2, l=L)
                        .unsqueeze(3).to_broadcast([1, 2, L, DSC, B]),
                    start=True, stop=True)
                nrmS = work.tile([128, FS], F32, tag="nrmS")
                nc.vector.tensor_tensor(out=nrmS[:], in0=stF[:],
                                        in1=ACSx[:, 0:FS], op=AL.mult)
                nc.vector.tensor_tensor(out=nrmS[:], in0=nrmS[:],
                                        in1=ACSx[:, FS:2 * FS], op=AL.add)

                # --- gen matmuls: predP[l*16+m*4 : +4] ---
                for l in range(L):
                    for m in range(DC):
                        o = predP[:, l * DC * B + m * B:l * DC * B + (m + 1) * B]
                        for k in range(DSC):
                            nc.tensor.matmul(
                                out=o, lhsT=av(f"gw_{l}_{k}_{m}"),
                                rhs=nrmS[:, l * DSC * B + k * B:l * DSC * B + (k + 1) * B],
                                start=(k == 0), stop=(k == DSC - 1))

                # --- LIF gen (ghat) ---
                gscl = work.tile([128, FD], F32, tag="gscl")
                nc.scalar.activation(gscl[:], ghF[:], AF.Copy, scale=BETA)
                gsum = work.tile([128, FD], F32, tag="gsum")
                nc.vector.tensor_tensor(out=gsum[:], in0=gscl[:], in1=predP, op=AL.add)
                pred = work.tile([128, FD], F32, tag="pred")
                nc.vector.tensor_tensor(out=pred[:], in0=gsum[:], in1=av("thrGF"),
                                        op=AL.is_gt)
                nc.vector.tensor_tensor(out=ghF[:], in0=gsum[:], in1=pred[:],
                                        op=AL.subtract)

                # --- err (bu chain) ---
                errF = work.tile([128, FD], F32, tag="errF")
                bu0 = seq4[:, :, :, bass.ds(sE, 1)]
                nc.vector.tensor_tensor(
                    out=errF[:, 0:16].rearrange("p (c b) -> p c b", c=DC).unsqueeze(3),
                    in0=bu0, in1=pred[:, 0:16].rearrange("p (c b) -> p c b", c=DC)
                    .unsqueeze(3), op=AL.subtract)
                if float(es[0]) != 1.0:
                    nc.vector.tensor_scalar_mul(errF[:, 0:16], errF[:, 0:16],
                                                float(es[0]))
                nc.vector.tensor_tensor(out=errF[:, 16:32], in0=errF[:, 0:16],
                                        in1=pred[:, 16:32], op=AL.subtract)
                if float(es[1]) != 1.0:
                    nc.vector.tensor_scalar_mul(errF[:, 16:32], errF[:, 16:32],
                                                float(es[1]))

                # --- stats of err (D norm) ---
                sqEF = work.tile([128, FD], F32, tag="sqEF")
                nc.scalar.activation(sqEF[:], errF[:], AF.Square, bias=av('zero128'))
                nc.tensor.matmul(out=mnEp, lhsT=av("vDn"), rhs=errF[:],
                                 start=True, stop=True)
                nc.tensor.matmul(out=sqEp, lhsT=av("vDp"), rhs=sqEF[:],
                                 start=True, stop=True)
                nc.vector.tensor_reduce(
                    out=mnE8.rearrange("p (l b) -> p l b", l=L),
                    in_=mnEp.rearrange("p (l c b) -> p l b c", l=L, c=DC),
                    axis=AX.X, op=AL.add)
                nc.vector.tensor_reduce(
                    out=sqE8.rearrange("p (l b) -> p l b", l=L),
                    in_=sqEp.rearrange("p (l c b) -> p l b c", l=L, c=DC),
                    axis=AX.X, op=AL.add)
                msqE = work.tile([1, 8], F32, tag="msqE")
                nc.vector.tensor_tensor(out=msqE[:], in0=mnE8, in1=mnE8, op=AL.mult)
                nc.vector.tensor_tensor(out=vE, in0=sqE8, in1=msqE[:], op=AL.subtract)
                nc.scalar.activation(sdE, vE, AF.Sqrt, bias=av('eps1')[0:1, :])
                nc.vector.reciprocal(rsE, sdE)
                nc.vector.tensor_tensor(out=cE, in0=mnE8, in1=rsE, op=AL.mult)
                nc.tensor.matmul(
                    out=ACEx, lhsT=onesRow[:, 0:128],
                    rhs=acs[0:1, 16:32].rearrange("p (a l b) -> p a l b", a=2, l=L)
                        .unsqueeze(3).to_broadcast([1, 2, L, DC, B]),
                    start=True, stop=True)
                nrmE = work.tile([128, FD], F32, tag="nrmE")
                nc.vector.tensor_tensor(out=nrmE[:], in0=errF[:],
                                        in1=ACEx[:, 0:FD], op=AL.mult)
                nc.vector.tensor_tensor(out=nrmE[:], in0=nrmE[:],
                                        in1=ACEx[:, FD:2 * FD], op=AL.add)

                # --- inf matmuls: infP[l*8+m*4 : +4] ---
                for l in range(L):
                    for m in range(DSC):
                        o = infP[:, l * DSC * B + m * B:l * DSC * B + (m + 1) * B]
                        for k in range(DC):
                            nc.tensor.matmul(
                                out=o, lhsT=av(f"iw_{l}_{k}_{m}"),
                                rhs=nrmE[:, l * DC * B + k * B:l * DC * B + (k + 1) * B],
                                start=(k == 0), stop=(k == DC - 1))

                # --- LIF inf + state update ---
                iscl = work.tile([128, FS], F32, tag="iscl")
                nc.scalar.activation(iscl[:], ihF[:], AF.Copy, scale=BETA)
                isum = work.tile([128, FS], F32, tag="isum")
                nc.vector.tensor_tensor(out=isum[:], in0=iscl[:], in1=infP, op=AL.add)
                upd = work.tile([128, FS], F32, tag="upd")
                nc.vector.tensor_tensor(out=upd[:], in0=isum[:], in1=av("thrIF"),
                                        op=AL.is_gt)
                nc.vector.tensor_tensor(out=ihF[:], in0=isum[:], in1=upd[:],
                                        op=AL.subtract)
                nc.vector.tensor_tensor(out=stF[:], in0=stF[:], in1=upd[:], op=AL.add)
                if snap:
                    nc.vector.tensor_copy(
                        out=hist4[:, :, :, bass.ds(sE, 1)],
                        in_=stF[:].rearrange("p (f b) -> p f b", f=L * DSC)
                        .unsqueeze(3))

            if krep > 1:
                with tc.For_i(0, krep) as _k:
                    emit_all()
            else:
                emit_all()

    nc.compile()
    return nc


# -------------------------------------------------------------------- driver
_CACHE = {}


def kernel(**inputs):
    fd = _fold(inputs)
    krep = int(os.environ.get("SNN_KREP", "1"))
    su = int(os.environ.get("SNN_SU", "16"))
    nc = build_program(fd, krep=krep, su=su)

    ids = np.asarray(inputs["input_ids"]).astype(np.int32).reshape(R, 1)
    emb = np.ascontiguousarray(np.asarray(inputs["emb"], np.float32))
    outw_pad, outb_pad = fd["outw_pad"], fd["outb_pad"]
    in_maps = []
    for c in range(NCORES):
        sl = outw_pad[:, c * VP:(c + 1) * VP]          # [512, VP]
        blk = np.ascontiguousarray(
            sl.reshape(DC, 128, VP).transpose(1, 0, 2))  # [128, DC, VP]
        in_maps.append({
            "emb": emb,
            "ids": ids,
            "outw": blk,
            "outb": np.ascontiguousarray(outb_pad[c * VP:(c + 1) * VP])[None, :],
        })
    r = run_bass_kernel_spmd(nc, in_maps, core_ids=list(range(NCORES)))
    full = np.concatenate([r.results[c]["out"] for c in range(NCORES)], axis=1)
    out = full[:, :V].reshape(B, S, V).astype(np.float32)
    _CACHE["nc"] = nc
    _CACHE["in_maps"] = in_maps
    return out
